# revision 3
# baseline (speedup 1.0000x reference)
"""CoverageAttention Trainium2 kernel v3 (8 NeuronCores, data-parallel over batch).

Math (graded inputs have alpha == 0, conv_b == 0, so the coverage branch is 0):
    pre[b,l,:] = A[b,l,:] @ Wa + hat_s_t[b] @ Ws          (A = i reshaped [B,L,C])
    e[b,l]     = tanh(pre[b,l,:]) @ v
    alpha'     = softmax(e, axis=1)
    out[b,:]   = sum_l alpha'[b,l] * A[b,l,:]

v4: window-outer schedule — per (b, w): 24 main matmuls (4 npc x 6 chunks,
4-6 pre PSUM banks), previous window's e-phase inserted between, then tanh.
The e-phase lags one window so the kernel tail is a single window's e-chain.
i tiles DMA'd in column quarters (chunk-major per quarter) so window 0's
matmuls start after ~1.2MB.

v3 structural changes over v2 (202us):
  * Main phase reuses each PE weight tile across ALL 7 windows (one
    LDWEIGHTS per (npc, chunk); 7 pre tiles live in 7 PSUM banks), instead
    of windows-of-3 groups.  Loop: npc -> c -> w.
  * The e-dot uses v REPLICATED across 128 PE columns (host-prepared
    vrep[p,k,m] = v[k*128+p]), so the 4 chained matmuls per window produce
    e already BROADCAST over 128 partitions in PSUM.  This removes the
    separate ones-broadcast matmul AND the PSUM->SBUF copy: ScalarE's exp
    reads the psum tile directly and writes the broadcast weights wbv
    (bf16, SBUF) consumed by the DVE context accumulation.
  * exp's accum_out gives the softmax denominator per window (row 0 used).
  * e-phase of batch b is interleaved into the main phase of batch b+1
    (two insertion gaps per npc sweep), so the single et PSUM bank WAR
    (et -> exp -> next window's et) hides under main-phase matmuls.
  * i is DMA'd unpadded (684 rows; 44-row chunk 5 + one-time gpsimd memset
    of the 84 garbage rows backing the full-width matmul reads).
"""

import numpy as np

B, C, H, W = 32, 684, 28, 112
L = H * W                      # 3136
Q, NP, N, KK, PAD = 256, 512, 256, 11, 5
NCORES = 8
BPC = B // NCORES              # 4 batch items per core
WIN = 448                      # 3136 = 7*448; 448*4B < 2KB PSUM bank
NWIN = L // WIN                # 7
NCH = 6                        # ceil(684/128)
LASTP = C - 5 * 128            # 44 real rows in chunk 5

_PROG = None
TRACE = False
LAST_RESULT = None


def _nparts(c):
    return 128 if c < 5 else LASTP


def _build_program():
    import concourse.bass as bass
    import concourse.bacc as bacc
    import concourse.tile as tile
    from concourse import mybir
    from contextlib import ExitStack

    f32 = mybir.dt.float32
    bf = mybir.dt.bfloat16

    nc = bacc.Bacc(trn_type="TRN2")

    fp8 = mybir.dt.float8e4
    DRM = mybir.MatmulPerfMode.DoubleRow

    i_d = nc.declare_dram_parameter("i", [BPC, 128, NCH, L], bf, isOutput=False)
    i8_d = nc.declare_dram_parameter("i8", [BPC, 128, NCH, L], fp8, isOutput=False)
    sp_d = nc.declare_dram_parameter("sproj", [128, BPC, 4], f32, isOutput=False)
    wa8_d = nc.declare_dram_parameter("wa8", [128, 3, 2, NP], fp8, isOutput=False)
    vr_d = nc.declare_dram_parameter("vrep", [128, 4, 128], bf, isOutput=False)
    u_ds = [nc.declare_dram_parameter(f"u{b}", [128, 8], f32, isOutput=True)
            for b in range(BPC)]

    TANH = mybir.ActivationFunctionType.Tanh
    EXP = mybir.ActivationFunctionType.Exp
    MULT = mybir.AluOpType.mult
    ADD = mybir.AluOpType.add

    with tile.TileContext(nc) as tc:
        with ExitStack() as ctx:
            singles = ctx.enter_context(tc.tile_pool(name="singles", bufs=1))
            thp = ctx.enter_context(tc.tile_pool(name="thp", bufs=12))
            wbvp = ctx.enter_context(tc.tile_pool(name="wbvp", bufs=4))
            scrp = ctx.enter_context(tc.tile_pool(name="scrp", bufs=2))
            up = ctx.enter_context(tc.tile_pool(name="up", bufs=16))
            prep = ctx.enter_context(tc.tile_pool(name="prep", bufs=6, space="PSUM"))
            etp = ctx.enter_context(tc.tile_pool(name="etp", bufs=2, space="PSUM"))

            # ---- static setup: few large DMAs (issue cost on the sync
            # engine is ~650ns per dma_start, so batch everything) ----
            # fp8 weights for the DoubleRow main matmul: wa8[p, j, s, n] =
            # quantized Wa[(2j+s)*128+p, n] * 16 (chunk pairs in the two
            # DoubleRow planes)
            wa8 = singles.tile([128, 3, 2, NP], fp8, tag="wa8")
            nc.sync.dma_start(out=wa8, in_=wa8_d[:, :, :, :])
            vrep = singles.tile([128, 4, 128], bf, tag="vrep")
            nc.sync.dma_start(out=vrep, in_=vr_d[:, :, :])
            sp_all = singles.tile([128, BPC, 4], f32, tag="sp")
            nc.sync.dma_start(out=sp_all, in_=sp_d[:, :, :])
            sp_sb = [sp_all[:, b, :] for b in range(BPC)]

            # i8: fp8 copy (x8, feedback-quantized) for the main matmul,
            # one [128, NCH, L] tile per batch, all resident.  The bf16
            # copy feeds the context stts and is streamed 2 batches deep.
            # DMAs are interleaved per batch (i8[b] then ibf[b]) so batch
            # 0's context data lands early; ibf skips the zero-pad rows of
            # chunk 5 (the stts only read the real 44).
            ibfp = ctx.enter_context(tc.tile_pool(name="ibfp", bufs=2))
            i8b = {}
            itb = {}
            for b in range(BPC):
                t8 = singles.tile([128, NCH, L], fp8, tag=f"i8_{b}")
                pieces = ((0, WIN), (WIN, L)) if b == 0 else ((0, L),)
                for c0, c1 in pieces:
                    nc.sync.dma_start(out=t8[:, :, c0:c1],
                                      in_=i8_d[b][:, :, c0:c1])
                i8b[b] = t8
                t = ibfp.tile([128, NCH, L], bf, tag="ibf", name=f"ibf{b}")
                # batches 2,3 reuse ibf buffers (WAR on batch b-2's stts);
                # issue those from the idle gpsimd queue so the wait cannot
                # block the sync engine's in-order DMA stream.
                eng = nc.sync if b < 2 else nc.gpsimd
                for c0, c1 in pieces:
                    eng.dma_start(out=t[:, 0:5, c0:c1],
                                  in_=i_d[b][:, 0:5, c0:c1])
                    eng.dma_start(out=t[0:LASTP, 5, c0:c1],
                                  in_=i_d[b][0:LASTP, 5, c0:c1])
                for c in range(NCH):
                    itb[b, c] = t[:, c, :]

            # per-batch state
            th = {}               # (b, w, npc) -> tanh tile
            uw = {}               # (b, c) -> [128, 8] f32 window-partials
            tacc = {}             # b -> [128, 8] f32 softmax denominators
            for b in range(BPC):
                tacc[b] = up.tile([128, 8], f32, tag="tacc", name=f"tacc{b}")
                for c in range(NCH):
                    uw[b, c] = up.tile([128, 8], f32, tag=f"uw{c}", name=f"uw_{b}_{c}")

            # context accumulation runs on GROUPS of windows (one wide stt
            # per chunk per group) to amortize the DVE per-op overhead; the
            # last batch uses finer trailing groups to keep the kernel tail
            # short.  GROUPS[b][w] = (group_index, group_start, group_len).
            GROUPS = {}
            for b in range(BPC):
                bounds = [(0, 4), (4, 3)] if b < BPC - 1 else [(0, 4), (4, 2), (6, 1)]
                GROUPS[b] = {}
                for gi, (gs, gl) in enumerate(bounds):
                    for w in range(gs, gs + gl):
                        GROUPS[b][w] = (gi, gs, gl)
                GROUPS[b]["n"] = len(bounds)
            wbvt = {}          # (b, gi) -> [128, gl*WIN] weight tile

            def emit_e_window(b, w):
                """e-dot + exp for window w; group-wide stt on the last
                window of each group."""
                et = etp.tile([128, WIN], f32, tag="et", name=f"et_{b}_{w}")
                ks = range(4) if w % 2 == 0 else range(3, -1, -1)
                for j, k in enumerate(ks):
                    nc.tensor.matmul(
                        et, vrep[:, k, :], th.pop((b, w, k)),
                        start=(j == 0), stop=(j == 3), skip_group_check=True)
                gi, gs, gl = GROUPS[b][w]
                side = w - gs
                if side == 0:
                    wbvt[b, gi] = wbvp.tile([128, gl * WIN], bf, tag="wbv",
                                            name=f"wbv_{b}_{gi}", bufs=3)
                wv = wbvt[b, gi]
                nc.scalar.activation(wv[:, side * WIN:(side + 1) * WIN], et, EXP,
                                     accum_out=tacc[b][:, w:w + 1])
                if side == gl - 1:
                    cw = gl * WIN
                    for c in range(NCH):
                        np_ = _nparts(c)
                        scr = scrp.tile([128, 4 * WIN], bf, tag="scr",
                                        name=f"scr_{b}_{w}_{c}")
                        nc.vector.scalar_tensor_tensor(
                            out=scr[0:np_, 0:cw],
                            in0=itb[b, c][0:np_, gs * WIN:gs * WIN + cw],
                            scalar=1.0,
                            in1=wv[0:np_, 0:cw],
                            op0=MULT, op1=MULT,
                            accum_out=uw[b, c][0:np_, gi:gi + 1])

            def emit_batch_out(b):
                ng = GROUPS[b]["n"]
                ua = up.tile([128, 8], f32, tag="ua", name=f"ua{b}")
                for c in range(NCH):
                    np_ = _nparts(c)
                    nc.vector.tensor_reduce(
                        out=ua[0:np_, c:c + 1], in_=uw[b, c][0:np_, 0:ng],
                        axis=mybir.AxisListType.X, op=ADD)
                # softmax denominator rides along as column 7 of u
                nc.vector.tensor_reduce(
                    out=ua[0:1, 7:8], in_=tacc[b][0:1, 0:NWIN],
                    axis=mybir.AxisListType.X, op=ADD)
                nc.sync.dma_start(out=u_ds[b][:, 0:8], in_=ua[:, 0:8])

            # window-outer schedule: per (b, w) compute all 4 npc pre tiles
            # (24 matmuls), then insert the PREVIOUS window's e-phase on the
            # PE, then tanh.  The e-phase thus lags one window and the only
            # tail is the final window's e-chain.
            prev_bw = None
            for b in range(BPC):
                for w in range(NWIN):
                    pres = []
                    for npc in range(4):
                        pre = prep.tile([128, WIN], f32, tag="pre",
                                        name=f"pre_{b}_{w}_{npc}")
                        for j in range(3):
                            nc.tensor.matmul(
                                pre, wa8[:, j, :, npc * 128:(npc + 1) * 128],
                                i8b[b][:, 2 * j:2 * j + 2,
                                       w * WIN:(w + 1) * WIN],
                                start=(j == 0), stop=(j == 2),
                                perf_mode=DRM, skip_group_check=True)
                        pres.append(pre)
                    if prev_bw is not None:
                        emit_e_window(*prev_bw)
                        if prev_bw[0] != b:            # batch boundary
                            emit_batch_out(prev_bw[0])
                    for npc in range(4):
                        t = thp.tile([128, WIN], bf, tag="th",
                                     name=f"th_{b}_{w}_{npc}")
                        nc.scalar.activation(t, pres[npc], TANH,
                                             bias=sp_sb[b][:, npc:npc + 1],
                                             scale=1.0 / 128.0)
                        th[b, w, npc] = t
                    prev_bw = (b, w)
            emit_e_window(*prev_bw)
            emit_batch_out(BPC - 1)

    _elide_redundant_ldweights(nc, mybir)
    nc.compile()
    return nc


def _elide_redundant_ldweights(nc, mybir):
    """Drop InstLdweights that reload the exact weights already resident in
    the PE array. Only sync-free loads are dropped."""
    removed = 0
    for blk in nc.main_func.blocks:
        insts = list(blk.instructions)
        loaded = None
        keep = []
        for inst in insts:
            if isinstance(inst, mybir.InstLdweights):
                sig = (str(inst.ins[0]), str(inst.tile_position),
                       str(inst.perf_mode), str(inst.is_transpose))
                si = inst.sync_info
                clean = si is None or (
                    len(si.on_wait) == 0 and len(si.on_update) == 0)
                if sig == loaded and clean:
                    removed += 1
                    continue
                loaded = sig
            keep.append(inst)
        if removed:
            blk.instructions[:] = keep
    return removed


def _get_program():
    global _PROG
    if _PROG is None:
        _PROG = _build_program()
    return _PROG


def _fp8_neighbors(x):
    """For finite f32 x (|x| < 240): the e4m3 lattice values lo <= x <= hi."""
    import ml_dtypes
    F8 = ml_dtypes.float8_e4m3
    ax = np.abs(x).astype(np.float32)
    q = ax.astype(F8)
    qf = q.astype(np.float32)
    b = q.view(np.uint8)
    up = (b + 1).view(F8).astype(np.float32)           # next |.| above qf
    dn = np.where(b > 0, (b - 1).astype(np.uint8).view(F8).astype(np.float32),
                  np.float32(0.0))
    hi_abs = np.where(ax > qf, up, qf)
    lo_abs = np.where(ax < qf, dn, qf)
    neg = x < 0
    lo = np.where(neg, -hi_abs, lo_abs)
    hi = np.where(neg, -lo_abs, hi_abs)
    return lo, hi


def _q8_feedback(X, wv, scale, axis):
    """Quantize X*scale to the e4m3 lattice, choosing the rounding direction
    along `axis` to cancel the wv-weighted running quantization error (so
    sum_k wv[k] * err[..., k] stays near zero).  Returns SCALED f32 values
    that are exactly representable in e4m3."""
    Xs = np.moveaxis(np.asarray(X, np.float32) * scale, axis, -1).copy()
    out = np.empty_like(Xs)
    s = np.zeros(Xs.shape[:-1], np.float32)
    for k in range(Xs.shape[-1]):
        x = Xs[..., k]
        lo, hi = _fp8_neighbors(x)
        e_lo = s + wv[k] * (lo - x)
        e_hi = s + wv[k] * (hi - x)
        pick = np.abs(e_hi) <= np.abs(e_lo)
        out[..., k] = np.where(pick, hi, lo)
        s = np.where(pick, e_hi, e_lo)
    return np.moveaxis(out, -1, axis)


def _reference_fallback(i, hat_s_t, alpha, conv_w, conv_b, Wa, Wf, Ws, v):
    b, c, h, w = i.shape
    Lq = h * w
    ap = np.pad(alpha[:, 0], ((0, 0), (PAD, PAD), (PAD, PAD)))
    F = np.zeros((b, Q, h, w), np.float32)
    for dy in range(KK):
        for dx in range(KK):
            patch = ap[:, dy:dy + h, dx:dx + w]
            F += conv_w[None, :, 0, dy, dx, None, None] * patch[:, None]
    F = F + conv_b[None, :, None, None]
    Fm = F.reshape(b, Q, Lq).transpose(0, 2, 1)
    A = i.reshape(b, c, Lq).transpose(0, 2, 1)
    pre = A @ Wa + Fm @ Wf + (hat_s_t @ Ws)[:, None, :]
    e = np.tanh(pre) @ v
    e = e - e.max(axis=1, keepdims=True)
    w_ = np.exp(e)
    aw = w_ / w_.sum(axis=1, keepdims=True)
    return np.einsum("bl,blc->bc", aw, A).astype(np.float32)


def kernel(i, hat_s_t, alpha, conv_w, conv_b, Wa, Wf, Ws, v):
    global LAST_RESULT
    i = np.ascontiguousarray(np.asarray(i, np.float32))
    hat_s_t = np.asarray(hat_s_t, np.float32)
    alpha = np.asarray(alpha, np.float32)
    conv_b = np.asarray(conv_b, np.float32)
    Wa = np.ascontiguousarray(np.asarray(Wa, np.float32))
    Ws = np.asarray(Ws, np.float32)
    v = np.ascontiguousarray(np.asarray(v, np.float32))

    if np.any(alpha) or np.any(conv_b):
        return _reference_fallback(i, hat_s_t, alpha,
                                   np.asarray(conv_w, np.float32),
                                   conv_b, Wa, np.asarray(Wf, np.float32), Ws, v)

    from concourse.bass_utils import run_bass_kernel_spmd
    import ml_dtypes
    hdt = ml_dtypes.bfloat16

    f8dt = ml_dtypes.float8_e4m3
    s_proj = (hat_s_t @ Ws).astype(np.float32)                         # [B, NP]
    # partition-major layouts so every DMA is a plain strided transfer
    sp_h = np.ascontiguousarray(
        s_proj.reshape(B, 4, 128).transpose(2, 0, 1))                  # [128,B,4]
    i_flat = np.zeros((B, NCH * 128, L), hdt)
    i_flat[:, :C, :] = i.reshape(B, C, L).astype(hdt)
    i_flat = np.ascontiguousarray(
        i_flat.reshape(B, NCH, 128, L).transpose(0, 2, 1, 3))          # [B,128,NCH,L]

    # fp8 main-matmul operands with error-feedback quantization:
    # Wa*16 quantized cancelling the v-weighted error along n'; then
    # i*8 quantized cancelling the h-weighted error along c, h = Wq @ v.
    wq_s = _q8_feedback(Wa, v, 16.0, axis=1)          # [C, NP], scaled by 16
    h = (wq_s / 16.0) @ v                             # [C]
    iq_s = _q8_feedback(i.reshape(B, C, L), h, 8.0, axis=1)   # scaled by 8
    wa8_h = np.zeros((NCH * 128, NP), f8dt)
    wa8_h[:C, :] = wq_s.astype(f8dt)
    wa8_h = np.ascontiguousarray(
        wa8_h.reshape(3, 2, 128, NP).transpose(2, 0, 1, 3))   # [128,3,2,NP]
    i8_flat = np.zeros((B, NCH * 128, L), f8dt)
    i8_flat[:, :C, :] = iq_s.astype(f8dt)
    i8_flat = np.ascontiguousarray(
        i8_flat.reshape(B, NCH, 128, L).transpose(0, 2, 1, 3))  # [B,128,NCH,L]
    # vrep[p, k, m] = v[k*128 + p] for all m (v replicated across PE columns)
    vrep = np.ascontiguousarray(np.broadcast_to(
        v.astype(hdt).reshape(4, 128).T[:, :, None], (128, 4, 128)))
    in_maps = []
    for k in range(NCORES):
        b0 = k * BPC
        in_maps.append({
            "i": np.ascontiguousarray(i_flat[b0:b0 + BPC]),
            "i8": np.ascontiguousarray(i8_flat[b0:b0 + BPC]),
            "sproj": np.ascontiguousarray(sp_h[:, b0:b0 + BPC, :]),
            "wa8": wa8_h,
            "vrep": vrep,
        })
    nc = _get_program()
    import time as _time
    t0 = _time.time()
    res = run_bass_kernel_spmd(nc, in_maps, list(range(NCORES)), trace=TRACE)
    res.exec_wall_s = _time.time() - t0
    LAST_RESULT = res
    out = np.empty((B, C), np.float32)
    for k in range(NCORES):
        for b in range(BPC):
            u = res.results[k][f"u{b}"]          # [128, 8]
            T = float(u[0, 7])
            chans = np.concatenate([u[:, c] for c in range(5)] + [u[:LASTP, 5]])
            out[k * BPC + b] = chans / T
    return out.astype(np.float32)


# revision 5
# speedup vs baseline: 1.0173x; 1.0173x over previous
"""CoverageAttention Trainium2 kernel (8 NeuronCores, data-parallel over batch).

Math (graded inputs have alpha == 0, conv_b == 0, so the coverage branch is 0):
    pre[b,l,:] = A[b,l,:] @ Wa + hat_s_t[b] @ Ws          (A = i reshaped [B,L,C])
    e[b,l]     = tanh(pre[b,l,:]) @ v
    alpha'     = softmax(e, axis=1)
    out[b,:]   = sum_l alpha'[b,l] * A[b,l,:]

Design (measured 129.1us vs the 202.5us bf16 v2 baseline; rel err 1.61e-2
vs the 2e-2 gate):

  * MAIN MATMUL IN FP8 (e4m3) WITH DoubleRow: each matmul contracts a
    256-row chunk-pair (two planes) at the bf16 per-column rate -> 2x
    FLOPs.  Accuracy survives the 2e-2 gate via ERROR-FEEDBACK
    QUANTIZATION on the host: Wa*16 is rounded along n' cancelling the
    v-weighted error sum per row, then i*8 is rounded along c cancelling
    the h-weighted error (h = Wq @ v).  This cuts the fp8 error from
    2.8e-2 (plain rounding, fails) to 1.6e-2.  The 1/128 descale rides on
    tanh's activation scale.
  * Window-outer schedule: per (batch, 448-column window): 12 DoubleRow
    matmuls (4 n'-chunks x 3 chunk-pairs) into 4 PSUM banks; the previous
    window's e-phase inserted between; then 4 tanh ops (s_proj rides as
    the per-partition activation bias).  The e-phase lags one window so
    the kernel tail is one window's chain.
  * e-dot: v REPLICATED across 128 PE columns (vrep[p,k,m] = v[k*128+p]),
    so 4 chained bf16 matmuls produce e already BROADCAST over the
    partitions; exp reads that PSUM tile directly and writes the
    broadcast softmax weights (bf16 SBUF) with the denominator via
    accum_out.  No separate broadcast matmul, no PSUM->SBUF copy.
    (The e-dot stays bf16: fp8 v or fp8 tanh would add ~1.3e-2 error.)
  * Context sum on DVE: one wide scalar_tensor_tensor per (chunk, window
    GROUP) -- groups of 4+3 windows (1+2+4 leading for batch 0 so the
    DVE starts early, 4+2+1 trailing for the last batch for a short
    tail) amortize the DVE per-op overhead; accum_out gives per-group partials, reduced
    per batch.  The context reads a SEPARATE bf16 copy of i (fp8 there
    would add ~2e-2 error on its own).
  * DMA: partition-major host layouts ([128, chunk, L] per batch), few
    large strided transfers issued in NEED-TIME order (batch 0's leading
    matmul columns first, then bias/v, then the rest interleaved with the
    context copy); the streamed bf16 tiles for batches 2,3 are issued
    from the idle gpsimd queue so their buffer-reuse waits cannot block
    the sync engine's in-order DMA stream.  bf16 copy skips the zero-pad
    rows of chunk 5.  Softmax denominator returns as column 7 of the u
    output (one DMA per batch).
"""

import numpy as np

B, C, H, W = 32, 684, 28, 112
L = H * W                      # 3136
Q, NP, N, KK, PAD = 256, 512, 256, 11, 5
NCORES = 8
BPC = B // NCORES              # 4 batch items per core
WIN = 448                      # 3136 = 7*448; 448*4B < 2KB PSUM bank
NWIN = L // WIN                # 7
NCH = 6                        # ceil(684/128)
LASTP = C - 5 * 128            # 44 real rows in chunk 5

_PROG = None
TRACE = False
LAST_RESULT = None


def _nparts(c):
    return 128 if c < 5 else LASTP


def _build_program():
    import concourse.bass as bass
    import concourse.bacc as bacc
    import concourse.tile as tile
    from concourse import mybir
    from contextlib import ExitStack

    f32 = mybir.dt.float32
    bf = mybir.dt.bfloat16

    nc = bacc.Bacc(trn_type="TRN2")

    fp8 = mybir.dt.float8e4
    DRM = mybir.MatmulPerfMode.DoubleRow

    i_d = nc.declare_dram_parameter("i", [BPC, 128, NCH, L], bf, isOutput=False)
    i8_d = nc.declare_dram_parameter("i8", [BPC, 128, NCH, L], fp8, isOutput=False)
    sp_d = nc.declare_dram_parameter("sproj", [128, BPC, 4], f32, isOutput=False)
    wa8_d = nc.declare_dram_parameter("wa8", [128, 3, 2, NP], fp8, isOutput=False)
    vr_d = nc.declare_dram_parameter("vrep", [128, 4, 128], bf, isOutput=False)
    u_ds = [nc.declare_dram_parameter(f"u{b}", [128, 8], f32, isOutput=True)
            for b in range(BPC)]

    TANH = mybir.ActivationFunctionType.Tanh
    EXP = mybir.ActivationFunctionType.Exp
    MULT = mybir.AluOpType.mult
    ADD = mybir.AluOpType.add

    with tile.TileContext(nc) as tc:
        with ExitStack() as ctx:
            singles = ctx.enter_context(tc.tile_pool(name="singles", bufs=1))
            thp = ctx.enter_context(tc.tile_pool(name="thp", bufs=12))
            wbvp = ctx.enter_context(tc.tile_pool(name="wbvp", bufs=4))
            scrp = ctx.enter_context(tc.tile_pool(name="scrp", bufs=2))
            up = ctx.enter_context(tc.tile_pool(name="up", bufs=16))
            prep = ctx.enter_context(tc.tile_pool(name="prep", bufs=6, space="PSUM"))
            etp = ctx.enter_context(tc.tile_pool(name="etp", bufs=2, space="PSUM"))

            # ---- static setup: few large DMAs (issue cost on the sync
            # engine is ~650ns per dma_start, so batch everything) ----
            # fp8 weights for the DoubleRow main matmul: wa8[p, j, s, n] =
            # quantized Wa[(2j+s)*128+p, n] * 16 (chunk pairs in the two
            # DoubleRow planes)
            wa8 = singles.tile([128, 3, 2, NP], fp8, tag="wa8")
            vrep = singles.tile([128, 4, 128], bf, tag="vrep")
            sp_all = singles.tile([128, BPC, 4], f32, tag="sp")
            sp_sb = [sp_all[:, b, :] for b in range(BPC)]
            nc.sync.dma_start(out=wa8, in_=wa8_d[:, :, :, :])

            # i8: fp8 copy (x8, feedback-quantized) for the main matmul,
            # one [128, NCH, L] tile per batch, all resident.  The bf16
            # copy feeds the context stts and is streamed 2 batches deep.
            # DMAs are interleaved per batch (i8[b] then ibf[b]) so batch
            # 0's context data lands early; ibf skips the zero-pad rows of
            # chunk 5 (the stts only read the real 44).
            ibfp = ctx.enter_context(tc.tile_pool(name="ibfp", bufs=2))
            i8b = {}
            itb = {}
            def dma_ibf(eng, t, b, c0, c1):
                eng.dma_start(out=t[:, 0:5, c0:c1], in_=i_d[b][:, 0:5, c0:c1])
                eng.dma_start(out=t[0:LASTP, 5, c0:c1],
                              in_=i_d[b][0:LASTP, 5, c0:c1])

            for b in range(BPC):
                t8 = singles.tile([128, NCH, L], fp8, tag=f"i8_{b}")
                i8b[b] = t8
                t = ibfp.tile([128, NCH, L], bf, tag="ibf", name=f"ibf{b}")
                for c in range(NCH):
                    itb[b, c] = t[:, c, :]
                if b == 0:
                    # need-time order: w0/w1 matmul columns, then the small
                    # bias/v tiles, then the rest, interleaved with the
                    # context copy's leading columns
                    nc.sync.dma_start(out=t8[:, :, 0:WIN],
                                      in_=i8_d[b][:, :, 0:WIN])
                    nc.sync.dma_start(out=t8[:, :, WIN:3 * WIN],
                                      in_=i8_d[b][:, :, WIN:3 * WIN])
                    nc.sync.dma_start(out=sp_all, in_=sp_d[:, :, :])
                    nc.sync.dma_start(out=vrep, in_=vr_d[:, :, :])
                    dma_ibf(nc.sync, t, b, 0, WIN)
                    nc.sync.dma_start(out=t8[:, :, 3 * WIN:L],
                                      in_=i8_d[b][:, :, 3 * WIN:L])
                    dma_ibf(nc.sync, t, b, WIN, 3 * WIN)
                    dma_ibf(nc.sync, t, b, 3 * WIN, L)
                else:
                    # batches 2,3 reuse ibf buffers (WAR on batch b-2's
                    # stts); issue those from the idle gpsimd queue so the
                    # wait cannot block the sync engine's in-order stream.
                    eng = nc.sync if b < 2 else nc.gpsimd
                    nc.sync.dma_start(out=t8[:, :, :], in_=i8_d[b][:, :, :])
                    dma_ibf(eng, t, b, 0, L)

            # per-batch state
            th = {}               # (b, w, npc) -> tanh tile
            uw = {}               # (b, c) -> [128, 8] f32 window-partials
            tacc = {}             # b -> [128, 8] f32 softmax denominators
            for b in range(BPC):
                tacc[b] = up.tile([128, 8], f32, tag="tacc", name=f"tacc{b}")
                for c in range(NCH):
                    uw[b, c] = up.tile([128, 8], f32, tag=f"uw{c}", name=f"uw_{b}_{c}")

            # context accumulation runs on GROUPS of windows (one wide stt
            # per chunk per group) to amortize the DVE per-op overhead; the
            # last batch uses finer trailing groups to keep the kernel tail
            # short.  GROUPS[b][w] = (group_index, group_start, group_len).
            GROUPS = {}
            for b in range(BPC):
                if b == 0:                  # fine leading groups: DVE starts early
                    bounds = [(0, 1), (1, 2), (3, 4)]
                elif b < BPC - 1:
                    bounds = [(0, 4), (4, 3)]
                else:                       # fine trailing groups: short tail
                    bounds = [(0, 4), (4, 2), (6, 1)]
                GROUPS[b] = {}
                for gi, (gs, gl) in enumerate(bounds):
                    for w in range(gs, gs + gl):
                        GROUPS[b][w] = (gi, gs, gl)
                GROUPS[b]["n"] = len(bounds)
            wbvt = {}          # (b, gi) -> [128, gl*WIN] weight tile

            def emit_e_window(b, w):
                """e-dot + exp for window w; group-wide stt on the last
                window of each group."""
                et = etp.tile([128, WIN], f32, tag="et", name=f"et_{b}_{w}")
                ks = range(4) if w % 2 == 0 else range(3, -1, -1)
                for j, k in enumerate(ks):
                    nc.tensor.matmul(
                        et, vrep[:, k, :], th.pop((b, w, k)),
                        start=(j == 0), stop=(j == 3), skip_group_check=True)
                gi, gs, gl = GROUPS[b][w]
                side = w - gs
                if side == 0:
                    wbvt[b, gi] = wbvp.tile([128, gl * WIN], bf, tag="wbv",
                                            name=f"wbv_{b}_{gi}", bufs=3)
                wv = wbvt[b, gi]
                nc.scalar.activation(wv[:, side * WIN:(side + 1) * WIN], et, EXP,
                                     accum_out=tacc[b][:, w:w + 1])
                if side == gl - 1:
                    cw = gl * WIN
                    for c in range(NCH):
                        np_ = _nparts(c)
                        scr = scrp.tile([128, 4 * WIN], bf, tag="scr",
                                        name=f"scr_{b}_{w}_{c}")
                        nc.vector.scalar_tensor_tensor(
                            out=scr[0:np_, 0:cw],
                            in0=itb[b, c][0:np_, gs * WIN:gs * WIN + cw],
                            scalar=1.0,
                            in1=wv[0:np_, 0:cw],
                            op0=MULT, op1=MULT,
                            accum_out=uw[b, c][0:np_, gi:gi + 1])

            def emit_batch_out(b):
                ng = GROUPS[b]["n"]
                ua = up.tile([128, 8], f32, tag="ua", name=f"ua{b}")
                for c in range(NCH):
                    np_ = _nparts(c)
                    nc.vector.tensor_reduce(
                        out=ua[0:np_, c:c + 1], in_=uw[b, c][0:np_, 0:ng],
                        axis=mybir.AxisListType.X, op=ADD)
                # softmax denominator rides along as column 7 of u
                nc.vector.tensor_reduce(
                    out=ua[0:1, 7:8], in_=tacc[b][0:1, 0:NWIN],
                    axis=mybir.AxisListType.X, op=ADD)
                nc.sync.dma_start(out=u_ds[b][:, 0:8], in_=ua[:, 0:8])

            # window-outer schedule: per (b, w) compute all 4 npc pre tiles
            # (24 matmuls), then insert the PREVIOUS window's e-phase on the
            # PE, then tanh.  The e-phase thus lags one window and the only
            # tail is the final window's e-chain.
            prev_bw = None
            for b in range(BPC):
                for w in range(NWIN):
                    pres = []
                    for npc in range(4):
                        pre = prep.tile([128, WIN], f32, tag="pre",
                                        name=f"pre_{b}_{w}_{npc}")
                        for j in range(3):
                            nc.tensor.matmul(
                                pre, wa8[:, j, :, npc * 128:(npc + 1) * 128],
                                i8b[b][:, 2 * j:2 * j + 2,
                                       w * WIN:(w + 1) * WIN],
                                start=(j == 0), stop=(j == 2),
                                perf_mode=DRM, skip_group_check=True)
                        pres.append(pre)
                    if prev_bw is not None:
                        emit_e_window(*prev_bw)
                        if prev_bw[0] != b:            # batch boundary
                            emit_batch_out(prev_bw[0])
                    for npc in range(4):
                        t = thp.tile([128, WIN], bf, tag="th",
                                     name=f"th_{b}_{w}_{npc}")
                        nc.scalar.activation(t, pres[npc], TANH,
                                             bias=sp_sb[b][:, npc:npc + 1],
                                             scale=1.0 / 128.0)
                        th[b, w, npc] = t
                    prev_bw = (b, w)
            emit_e_window(*prev_bw)
            emit_batch_out(BPC - 1)

    _elide_redundant_ldweights(nc, mybir)
    nc.compile()
    return nc


def _elide_redundant_ldweights(nc, mybir):
    """Drop InstLdweights that reload the exact weights already resident in
    the PE array. Only sync-free loads are dropped."""
    removed = 0
    for blk in nc.main_func.blocks:
        insts = list(blk.instructions)
        loaded = None
        keep = []
        for inst in insts:
            if isinstance(inst, mybir.InstLdweights):
                sig = (str(inst.ins[0]), str(inst.tile_position),
                       str(inst.perf_mode), str(inst.is_transpose))
                si = inst.sync_info
                clean = si is None or (
                    len(si.on_wait) == 0 and len(si.on_update) == 0)
                if sig == loaded and clean:
                    removed += 1
                    continue
                loaded = sig
            keep.append(inst)
        if removed:
            blk.instructions[:] = keep
    return removed


def _get_program():
    global _PROG
    if _PROG is None:
        _PROG = _build_program()
    return _PROG


def _fp8_neighbors(x):
    """For finite f32 x (|x| < 240): the e4m3 lattice values lo <= x <= hi."""
    import ml_dtypes
    F8 = ml_dtypes.float8_e4m3
    ax = np.abs(x).astype(np.float32)
    q = ax.astype(F8)
    qf = q.astype(np.float32)
    b = q.view(np.uint8)
    up = (b + 1).view(F8).astype(np.float32)           # next |.| above qf
    dn = np.where(b > 0, (b - 1).astype(np.uint8).view(F8).astype(np.float32),
                  np.float32(0.0))
    hi_abs = np.where(ax > qf, up, qf)
    lo_abs = np.where(ax < qf, dn, qf)
    neg = x < 0
    lo = np.where(neg, -hi_abs, lo_abs)
    hi = np.where(neg, -lo_abs, hi_abs)
    return lo, hi


def _q8_feedback(X, wv, scale, axis):
    """Quantize X*scale to the e4m3 lattice, choosing the rounding direction
    along `axis` to cancel the wv-weighted running quantization error (so
    sum_k wv[k] * err[..., k] stays near zero).  Returns SCALED f32 values
    that are exactly representable in e4m3."""
    Xs = np.moveaxis(np.asarray(X, np.float32) * scale, axis, -1).copy()
    out = np.empty_like(Xs)
    s = np.zeros(Xs.shape[:-1], np.float32)
    for k in range(Xs.shape[-1]):
        x = Xs[..., k]
        lo, hi = _fp8_neighbors(x)
        e_lo = s + wv[k] * (lo - x)
        e_hi = s + wv[k] * (hi - x)
        pick = np.abs(e_hi) <= np.abs(e_lo)
        out[..., k] = np.where(pick, hi, lo)
        s = np.where(pick, e_hi, e_lo)
    return np.moveaxis(out, -1, axis)


def _reference_fallback(i, hat_s_t, alpha, conv_w, conv_b, Wa, Wf, Ws, v):
    b, c, h, w = i.shape
    Lq = h * w
    ap = np.pad(alpha[:, 0], ((0, 0), (PAD, PAD), (PAD, PAD)))
    F = np.zeros((b, Q, h, w), np.float32)
    for dy in range(KK):
        for dx in range(KK):
            patch = ap[:, dy:dy + h, dx:dx + w]
            F += conv_w[None, :, 0, dy, dx, None, None] * patch[:, None]
    F = F + conv_b[None, :, None, None]
    Fm = F.reshape(b, Q, Lq).transpose(0, 2, 1)
    A = i.reshape(b, c, Lq).transpose(0, 2, 1)
    pre = A @ Wa + Fm @ Wf + (hat_s_t @ Ws)[:, None, :]
    e = np.tanh(pre) @ v
    e = e - e.max(axis=1, keepdims=True)
    w_ = np.exp(e)
    aw = w_ / w_.sum(axis=1, keepdims=True)
    return np.einsum("bl,blc->bc", aw, A).astype(np.float32)


def kernel(i, hat_s_t, alpha, conv_w, conv_b, Wa, Wf, Ws, v):
    global LAST_RESULT
    i = np.ascontiguousarray(np.asarray(i, np.float32))
    hat_s_t = np.asarray(hat_s_t, np.float32)
    alpha = np.asarray(alpha, np.float32)
    conv_b = np.asarray(conv_b, np.float32)
    Wa = np.ascontiguousarray(np.asarray(Wa, np.float32))
    Ws = np.asarray(Ws, np.float32)
    v = np.ascontiguousarray(np.asarray(v, np.float32))

    if np.any(alpha) or np.any(conv_b):
        return _reference_fallback(i, hat_s_t, alpha,
                                   np.asarray(conv_w, np.float32),
                                   conv_b, Wa, np.asarray(Wf, np.float32), Ws, v)

    from concourse.bass_utils import run_bass_kernel_spmd
    import ml_dtypes
    hdt = ml_dtypes.bfloat16

    f8dt = ml_dtypes.float8_e4m3
    s_proj = (hat_s_t @ Ws).astype(np.float32)                         # [B, NP]
    # partition-major layouts so every DMA is a plain strided transfer
    sp_h = np.ascontiguousarray(
        s_proj.reshape(B, 4, 128).transpose(2, 0, 1))                  # [128,B,4]
    i_flat = np.zeros((B, NCH * 128, L), hdt)
    i_flat[:, :C, :] = i.reshape(B, C, L).astype(hdt)
    i_flat = np.ascontiguousarray(
        i_flat.reshape(B, NCH, 128, L).transpose(0, 2, 1, 3))          # [B,128,NCH,L]

    # fp8 main-matmul operands with error-feedback quantization:
    # Wa*16 quantized cancelling the v-weighted error along n'; then
    # i*8 quantized cancelling the h-weighted error along c, h = Wq @ v.
    wq_s = _q8_feedback(Wa, v, 16.0, axis=1)          # [C, NP], scaled by 16
    h = (wq_s / 16.0) @ v                             # [C]
    iq_s = _q8_feedback(i.reshape(B, C, L), h, 8.0, axis=1)   # scaled by 8
    wa8_h = np.zeros((NCH * 128, NP), f8dt)
    wa8_h[:C, :] = wq_s.astype(f8dt)
    wa8_h = np.ascontiguousarray(
        wa8_h.reshape(3, 2, 128, NP).transpose(2, 0, 1, 3))   # [128,3,2,NP]
    i8_flat = np.zeros((B, NCH * 128, L), f8dt)
    i8_flat[:, :C, :] = iq_s.astype(f8dt)
    i8_flat = np.ascontiguousarray(
        i8_flat.reshape(B, NCH, 128, L).transpose(0, 2, 1, 3))  # [B,128,NCH,L]
    # vrep[p, k, m] = v[k*128 + p] for all m (v replicated across PE columns)
    vrep = np.ascontiguousarray(np.broadcast_to(
        v.astype(hdt).reshape(4, 128).T[:, :, None], (128, 4, 128)))
    in_maps = []
    for k in range(NCORES):
        b0 = k * BPC
        in_maps.append({
            "i": np.ascontiguousarray(i_flat[b0:b0 + BPC]),
            "i8": np.ascontiguousarray(i8_flat[b0:b0 + BPC]),
            "sproj": np.ascontiguousarray(sp_h[:, b0:b0 + BPC, :]),
            "wa8": wa8_h,
            "vrep": vrep,
        })
    nc = _get_program()
    import time as _time
    t0 = _time.time()
    res = run_bass_kernel_spmd(nc, in_maps, list(range(NCORES)), trace=TRACE)
    res.exec_wall_s = _time.time() - t0
    LAST_RESULT = res
    out = np.empty((B, C), np.float32)
    for k in range(NCORES):
        for b in range(BPC):
            u = res.results[k][f"u{b}"]          # [128, 8]
            T = float(u[0, 7])
            chans = np.concatenate([u[:, c] for c in range(5)] + [u[:LASTP, 5]])
            out[k * BPC + b] = chans / T
    return out.astype(np.float32)
